# revision 7
# baseline (speedup 1.0000x reference)
"""Trainium2 Bass kernel for nn_Attention_28930899706081 (sparse_attention).

Reference computation:
  k1 = l2norm_c(Wqk @ fmap1), k2 = l2norm_c(Wqk @ fmap2), q = l2norm_c(Wqk @ dmap)
  sim_i = q^T k_i per batch  -> [b, n, n] with n = h*w = 4096
  attn_i = softmax(sim_i, axis=-1)[:, None]  -> [b, 1, n, n]
  returns (attn1, attn2)

Sharding: 8 cores; core i handles batch b = i//4 and query-row block r = i%4
(1024 of 4096 rows). Each core recomputes the full normalized K for its batch
and its row block of both sims + softmax.

Schedule: ScalarE runs ONLY the 32 softmax exps (the 62.8us floor) plus one
table load at t=0. Column inverse-norms are computed compactly ([128, ng]
layout via per-group ones-matmuls with interleaved column groups), rsqrt'd
with a Newton iteration on VectorE (bit-trick seed), flattened to [1, xch]
by a gpsimd DMA (the interleaved grouping makes the partition-major flatten
order come out right), and partition-broadcast for the column normalize.
PSUM is one shared 2x[128,2048] pool rotated between projection chunks and
sim tiles so the pipeline is exp-paced end to end.
"""

import numpy as np
import ml_dtypes

B, C, H, W, D = 2, 256, 64, 64, 128
N = H * W  # 4096
QBLK = N // 4  # 1024 query rows per core
N_CORES = 8
CH = 2048   # sim/exp chunk (one PSUM tile)
PCH = 512   # matmul free-dim chunk (one PSUM bank)

_cached = {}


def _build():
    import concourse.mybir as mybir
    import concourse.tile as tile
    from concourse import bacc
    from contextlib import ExitStack

    f32 = mybir.dt.float32
    bf16 = mybir.dt.bfloat16
    i32 = mybir.dt.int32
    AF = mybir.ActivationFunctionType
    ALU = mybir.AluOpType

    nc = bacc.Bacc(
        "TRN2",
        target_bir_lowering=False,
        debug=False,
        enable_asserts=False,
        num_devices=N_CORES,
    )

    f1_ext = nc.dram_tensor("f1", [C, N], bf16, kind="ExternalInput").ap()
    f2_ext = nc.dram_tensor("f2", [C, N], bf16, kind="ExternalInput").ap()
    xq_ext = nc.dram_tensor("xq", [C, QBLK], bf16, kind="ExternalInput").ap()
    wqkT_ext = nc.dram_tensor("wqkT", [C, D], bf16, kind="ExternalInput").ap()
    out_ext = nc.dram_tensor("out", [2, QBLK, N], bf16, kind="ExternalOutput").ap()

    with tile.TileContext(nc) as tc, ExitStack() as ctx:
        consts = ctx.enter_context(tc.tile_pool(name="consts", bufs=1))
        xin = ctx.enter_context(tc.tile_pool(name="xin", bufs=14))
        ybf_p = ctx.enter_context(tc.tile_pool(name="ybf", bufs=2))
        ysq_p = ctx.enter_context(tc.tile_pool(name="ysq", bufs=2))
        nwt_p = ctx.enter_context(tc.tile_pool(name="nwt", bufs=2))
        flat_p = ctx.enter_context(tc.tile_pool(name="flat", bufs=2))
        kn_p = ctx.enter_context(tc.tile_pool(name="kn", bufs=1))
        e_p = ctx.enter_context(tc.tile_pool(name="epool", bufs=4))
        attn_p = ctx.enter_context(tc.tile_pool(name="attn", bufs=4))
        stat_p = ctx.enter_context(tc.tile_pool(name="stat", bufs=8))

        # constants + the ONLY activation table load of the kernel (Exp)
        wqkT_sb = [
            consts.tile([128, D], bf16, tag=f"wqkT{k}", name=f"wqkT{k}")
            for k in range(2)
        ]
        nc.gpsimd.dma_start(out=wqkT_sb[0][:], in_=wqkT_ext[0:128, :])
        nc.gpsimd.dma_start(out=wqkT_sb[1][:], in_=wqkT_ext[128:256, :])
        ones_col = consts.tile([128, 1], bf16, tag="ones", name="ones")
        nc.vector.memset(ones_col[:], 1.0)
        ones_row = consts.tile([1, 128], bf16, tag="onesr", name="onesr")
        nc.vector.memset(ones_row[:], 1.0)
        warm = consts.tile([128, 1], f32, tag="warm", name="warm")
        nc.scalar.activation(out=warm[:], in_=ones_col[:], func=AF.Exp)

        # -------- front-load all input DMAs (sync queue, in priority order)
        def load_x(x_ext, ncols, xch, tagbase):
            tiles = []
            for h0 in range(0, ncols, xch):
                lo = xin.tile([128, xch], bf16, tag="xi", name=f"{tagbase}lo{h0}")
                hi = xin.tile([128, xch], bf16, tag="xi", name=f"{tagbase}hi{h0}")
                nc.sync.dma_start(out=lo[:], in_=x_ext[0:128, h0 : h0 + xch])
                nc.sync.dma_start(out=hi[:], in_=x_ext[128:256, h0 : h0 + xch])
                tiles.append((h0, xch, lo, hi))
            return tiles

        xq_t = load_x(xq_ext, QBLK, 1024, "xq")
        f1_t = load_x(f1_ext, N, 1024, "f1")
        f2_t = load_x(f2_ext, N, 2048, "f2")

        psum = ctx.enter_context(
            tc.tile_pool(name="P", bufs=2, space="PSUM")
        )

        def phase_a_chunk(kn, h0, xch, x_lo, x_hi):
            """project chunk + compact inverse-norm + column normalize."""
            ng = xch // 128
            ps = psum.tile([128, CH], f32, tag="P", name=f"ps_{kn.name}_{h0}")
            for c in range(xch // PCH):
                sl = slice(c * PCH, (c + 1) * PCH)
                nc.tensor.matmul(ps[:, sl], wqkT_sb[0][:], x_lo[:, sl],
                                 start=True, stop=False)
            for c in range(xch // PCH):
                sl = slice(c * PCH, (c + 1) * PCH)
                nc.tensor.matmul(ps[:, sl], wqkT_sb[1][:], x_hi[:, sl],
                                 start=False, stop=True)
            y_bf = ybf_p.tile([128, xch], bf16, tag="ybf", name=f"y_{kn.name}_{h0}")
            nc.vector.tensor_copy(y_bf[:], ps[:, 0:xch])
            ysq = ysq_p.tile([128, xch], bf16, tag="ysq", name=f"sq_{kn.name}_{h0}")
            nc.vector.tensor_mul(ysq[:], y_bf[:], y_bf[:])
            # compact col-norms: group g holds cols {m*ng + g}; n2_t[p, g] =
            # |col p*ng+g|^2 so the partition-major flatten below is in order
            for g in range(ng):
                lhsT = ysq[:, g : xch : ng]
                nc.tensor.matmul(ps[:, g : g + 1], lhsT, ones_col[:],
                                 start=True, stop=True)
            n2c = nwt_p.tile([128, 16], f32, tag="n2c", name=f"n2_{kn.name}_{h0}")
            nc.vector.tensor_copy(n2c[:, 0:ng], ps[:, 0:ng])
            # newton rsqrt on the compact tile
            nsl = slice(0, ng)
            ish = nwt_p.tile([128, 16], i32, tag="ish", name=f"is_{kn.name}_{h0}")
            nc.vector.tensor_scalar(
                out=ish[:, nsl], in0=n2c[:, nsl].bitcast(i32), scalar1=1,
                scalar2=0xFFFFFFFF, op0=ALU.arith_shift_right,
                op1=ALU.bitwise_xor)
            seed = nwt_p.tile([128, 16], i32, tag="seed", name=f"sd_{kn.name}_{h0}")
            nc.vector.tensor_scalar_add(seed[:, nsl], ish[:, nsl], 0x5F3759E0)
            r0 = seed[:, nsl].bitcast(f32)
            t0 = nwt_p.tile([128, 16], f32, tag="t0", name=f"t0_{kn.name}_{h0}")
            t1 = nwt_p.tile([128, 16], f32, tag="t1", name=f"t1_{kn.name}_{h0}")
            r1 = nwt_p.tile([128, 16], f32, tag="r1", name=f"r1_{kn.name}_{h0}")
            rkt = nwt_p.tile([128, 16], bf16, tag="rkt", name=f"rk_{kn.name}_{h0}")
            nc.vector.tensor_mul(t0[:, nsl], r0, n2c[:, nsl])
            nc.vector.tensor_mul(t1[:, nsl], t0[:, nsl], r0)
            nc.vector.tensor_scalar(
                out=t1[:, nsl], in0=t1[:, nsl], scalar1=-0.5, scalar2=1.5,
                op0=ALU.mult, op1=ALU.add)
            nc.vector.tensor_mul(r1[:, nsl], t1[:, nsl], r0)
            nc.vector.tensor_mul(t0[:, nsl], r1[:, nsl], n2c[:, nsl])
            nc.vector.tensor_mul(t1[:, nsl], t0[:, nsl], r1[:, nsl])
            nc.vector.tensor_scalar(
                out=t1[:, nsl], in0=t1[:, nsl], scalar1=-0.5, scalar2=1.5,
                op0=ALU.mult, op1=ALU.add)
            nc.vector.tensor_mul(rkt[:, nsl], t1[:, nsl], r1[:, nsl])
            # flatten [128, ng] -> [1, xch]: flat[p*ng+g] = rk[col p*ng+g]
            flat = flat_p.tile([1, CH], bf16, tag="flat", name=f"fl_{kn.name}_{h0}")
            nc.gpsimd.dma_start(out=flat[0:1, 0:xch], in_=rkt[:, nsl])
            # broadcast rk across partitions with a K=1 ones-row matmul,
            # reusing the proj psum tile (free by now)
            for c in range(xch // PCH):
                sl = slice(c * PCH, (c + 1) * PCH)
                nc.tensor.matmul(ps[:, sl], ones_row[0:1, :], flat[0:1, sl],
                                 start=True, stop=True)
            nc.vector.tensor_mul(kn[:, h0 : h0 + xch], y_bf[:], ps[:, 0:xch])

        qn = kn_p.tile([128, QBLK], bf16, tag="qn", name="qn")
        k1n = kn_p.tile([128, N], bf16, tag="k1n", name="k1n")
        k2n = kn_p.tile([128, N], bf16, tag="k2n", name="k2n")

        for (h0, xch, lo, hi) in xq_t:
            phase_a_chunk(qn, h0, xch, lo, hi)
        for (h0, xch, lo, hi) in f1_t:
            phase_a_chunk(k1n, h0, xch, lo, hi)

        # -------- phase B --------
        def sim_chunk(kn, s, t, j, stile):
            """sim + exp for row tile t, column chunk j of map kn."""
            lhsT = qn[:, t * 128 : (t + 1) * 128]
            ps = psum.tile([128, CH], f32, tag="P", name=f"sim{s}_{t}_{j}")
            for c in range(CH // PCH):
                csl = slice(j * CH + c * PCH, j * CH + (c + 1) * PCH)
                nc.tensor.matmul(ps[:, c * PCH : (c + 1) * PCH], lhsT,
                                 kn[:, csl], start=True, stop=True)
            e = e_p.tile([128, CH], bf16, tag="e", name=f"e{s}_{t}_{j}")
            nc.scalar.activation(out=e[:], in_=ps[:], func=AF.Exp,
                                 accum_out=stile[:, j : j + 1])
            return e

        def finish_tile(s, t, stile, e_chunks):
            ssum = stat_p.tile([128, 1], f32, tag="ssum", name=f"zs{s}_{t}")
            nc.vector.reduce_sum(ssum[:], stile[:], axis=mybir.AxisListType.X)
            recip = stat_p.tile([128, 1], f32, tag="recip", name=f"rc{s}_{t}")
            nc.vector.reciprocal(recip[:], ssum[:])
            for j, e in enumerate(e_chunks):
                attn = attn_p.tile([128, CH], bf16, tag="attn", name=f"a{s}_{t}_{j}")
                nc.vector.tensor_scalar_mul(attn[:], e[:], recip[:])
                nc.sync.dma_start(
                    out=out_ext[s, t * 128 : (t + 1) * 128,
                                j * CH : (j + 1) * CH],
                    in_=attn[:],
                )

        # k1 tiles with k2's phase A interleaved into the gaps
        k2_iter = iter(f2_t)

        def emit_k2_chunk():
            nxt = next(k2_iter, None)
            if nxt is not None:
                h0, xch, lo, hi = nxt
                phase_a_chunk(k2n, h0, xch, lo, hi)

        stiles = {}
        for t in range(QBLK // 128):
            stile = stat_p.tile([128, 2], f32, tag="stile", name=f"st0_{t}")
            e0 = sim_chunk(k1n, 0, t, 0, stile)
            if t in (0, 3):
                emit_k2_chunk()
            e1 = sim_chunk(k1n, 0, t, 1, stile)
            finish_tile(0, t, stile, [e0, e1])
        for t in range(QBLK // 128):
            stile = stat_p.tile([128, 2], f32, tag="stile", name=f"st1_{t}")
            e0 = sim_chunk(k2n, 1, t, 0, stile)
            e1 = sim_chunk(k2n, 1, t, 1, stile)
            finish_tile(1, t, stile, [e0, e1])

    nc.compile()
    return nc


def _get_nc():
    if "nc" not in _cached:
        _cached["nc"] = _build()
    return _cached["nc"]


def _in_maps(fmap1, fmap2, dmap, Wqk):
    bf = ml_dtypes.bfloat16
    f1r = np.asarray(fmap1, dtype=np.float32).reshape(B, C, N)
    f2r = np.asarray(fmap2, dtype=np.float32).reshape(B, C, N)
    dqr = np.asarray(dmap, dtype=np.float32).reshape(B, C, N)
    wT = np.ascontiguousarray(np.asarray(Wqk, dtype=np.float32).T).astype(bf)

    in_maps = []
    for i in range(N_CORES):
        b, r = divmod(i, 4)
        in_maps.append(
            {
                "f1": np.ascontiguousarray(f1r[b]).astype(bf),
                "f2": np.ascontiguousarray(f2r[b]).astype(bf),
                "xq": np.ascontiguousarray(
                    dqr[b][:, r * QBLK : (r + 1) * QBLK]
                ).astype(bf),
                "wqkT": wT,
            }
        )
    return in_maps


def kernel(fmap1, fmap2, dmap, Wqk):
    from concourse.bass_utils import run_bass_kernel_spmd

    in_maps = _in_maps(fmap1, fmap2, dmap, Wqk)
    nc = _get_nc()
    res = run_bass_kernel_spmd(nc, in_maps, core_ids=list(range(N_CORES)))
    _cached["last_result"] = res

    attn1 = np.empty((B, 1, N, N), dtype=np.float32)
    attn2 = np.empty((B, 1, N, N), dtype=np.float32)
    for i in range(N_CORES):
        b, r = divmod(i, 4)
        o = res.results[i]["out"]
        attn1[b, 0, r * QBLK : (r + 1) * QBLK, :] = o[0].astype(np.float32)
        attn2[b, 0, r * QBLK : (r + 1) * QBLK, :] = o[1].astype(np.float32)
    return (attn1, attn2)
